# revision 3
# baseline (speedup 1.0000x reference)
"""MatchingNetwork forward on 8 TRN2 NeuronCores.

Computation (reference):
    s_emb = l2norm(support @ W + b); q_emb = l2norm(query @ W + b)
    out = softmax(q_emb @ s_emb.T, axis=1) @ one_hot(labels, 64)

Strategy: data-parallel over query rows (1024/core), support replicated.
Host passes pre-transposed S^T / Q^T so every matmul has its contraction
dim on partitions (no on-device transposes). Attention is fused: per
512-query block we accumulate P^T[c,i] = sum_j OH[j,c]*exp(logit[j,i])
over support chunks; the softmax denominator is the column sum of P^T
(each one-hot row sums to 1), so the full attention matrix never exists.
"""

import sys

if "/opt/trn_rl_repo" not in sys.path:
    sys.path.insert(0, "/opt/trn_rl_repo")

import ml_dtypes
import numpy as np

import concourse.mybir as mybir
import concourse.tile as tile
from concourse import bacc, bass_utils

N_CORES = 8
NS, NQ, IND, D, C = 4096, 8192, 1024, 512, 64
NQC = NQ // N_CORES  # queries per core
KC = IND // 128      # 8 contraction chunks
DC = D // 128        # 4 embedding-dim chunks
JBLK = 512           # support/query columns per encode block
NJB = NS // JBLK     # 8 support encode blocks
NJC = NS // 128      # 32 support chunks in attention
NIB = NQC // 512     # 2 query blocks per core

F32 = mybir.dt.float32
F32R = mybir.dt.float32r
BF16 = mybir.dt.bfloat16


def _emit(nc, tc, s_t, q_t, w, b, oh, out):
    FT = mybir.ActivationFunctionType
    import contextlib

    with contextlib.ExitStack() as ctx:
        const = ctx.enter_context(tc.tile_pool(name="const", bufs=1))

        w_sb = const.tile([128, KC, D], F32R)
        nc.sync.dma_start(w_sb[:], w.rearrange("(kc p) d -> p kc d", p=128))
        b_sb = const.tile([128, DC], F32)
        nc.sync.dma_start(b_sb[:], b.rearrange("(dc p) -> p dc", p=128))
        oh_sb = const.tile([128, NJC, C], BF16)
        nc.sync.dma_start(oh_sb[:], oh.rearrange("(jc p) c -> p jc c", p=128))
        ones_f32 = const.tile([128, 128], F32)
        nc.vector.memset(ones_f32[:], 1.0)
        ones_col = const.tile([128, 1], F32R)
        nc.scalar.copy(ones_col[:], ones_f32[:, 0:1])
        ones_row = const.tile([1, 128], F32R)
        nc.scalar.copy(ones_row[:], ones_f32[0:1, :])

        semb = const.tile([128, DC, NS], F32R)   # normalized S_emb^T, resident
        qemb = const.tile([128, DC, NQC], F32R)  # normalized Q_emb^T, resident

        def encode(x_t, n_cols, emb):
            """emb[:, dc, :] = l2norm-columns of (W^T @ x + b)."""
            with tc.tile_pool(name="enc_load", bufs=12) as loadp, \
                 tc.tile_pool(name="enc_work", bufs=3) as work, \
                 tc.tile_pool(name="enc_ps", bufs=2, space="PSUM") as psum, \
                 tc.tile_pool(name="enc_acc", bufs=2, space="PSUM") as psacc:
                xr = x_t.rearrange("(kc p) n -> p kc n", p=128)
                for jb in range(n_cols // JBLK):
                    js = slice(jb * JBLK, (jb + 1) * JBLK)
                    xt = []
                    for kc in range(KC):
                        t = loadp.tile([128, JBLK], F32R, tag="xt")
                        nc.sync.dma_start(t[:], xr[:, kc, js])
                        xt.append(t)
                    nrm_ps = psacc.tile([1, JBLK], F32, tag="nrm")
                    for dc in range(DC):
                        ps = psum.tile([128, JBLK], F32, tag="enc")
                        for kc in range(KC):
                            nc.tensor.matmul(
                                ps[:],
                                w_sb[:, kc, dc * 128:(dc + 1) * 128],
                                xt[kc][:],
                                start=(kc == 0), stop=(kc == KC - 1),
                            )
                        bias = b_sb[:, dc:dc + 1]
                        nc.scalar.activation(emb[:, dc, js], ps[:], FT.Identity, bias=bias)
                        sq = work.tile([128, JBLK], F32R, tag="sq")
                        nc.scalar.activation(sq[:], ps[:], FT.Square, bias=bias)
                        nc.tensor.matmul(
                            nrm_ps[:], ones_col[:], sq[:],
                            start=(dc == 0), stop=(dc == DC - 1),
                        )
                    nr = work.tile([1, JBLK], F32R, tag="nr")
                    nc.scalar.copy(nr[:], nrm_ps[:])
                    rep_ps = psum.tile([128, JBLK], F32, tag="rep")
                    nc.tensor.matmul(rep_ps[:], ones_row[:],
                                     nr[:], start=True, stop=True)
                    srep = work.tile([128, JBLK], F32, tag="srep")
                    nc.scalar.activation(srep[:], rep_ps[:], FT.Sqrt)
                    irep = work.tile([128, JBLK], F32, tag="irep")
                    nc.vector.reciprocal(irep[:], srep[:])
                    for dc in range(DC):
                        sl = emb[:, dc, js]
                        nc.vector.tensor_mul(sl, sl, irep[:])

        encode(s_t, NS, semb)
        encode(q_t, NQC, qemb)

        with tc.tile_pool(name="att_work", bufs=3) as work, \
             tc.tile_pool(name="att_lg", bufs=3, space="PSUM") as pslg, \
             tc.tile_pool(name="att_acc", bufs=2, space="PSUM") as psacc, \
             tc.tile_pool(name="att_sm", bufs=1, space="PSUM") as pssm:
            for ib in range(NIB):
                isl = slice(ib * 512, (ib + 1) * 512)
                p_ps = psacc.tile([C, 512], F32, tag="pacc")
                for jc in range(NJC):
                    lg = pslg.tile([128, 512], F32, tag="lg")
                    for dc in range(DC):
                        nc.tensor.matmul(
                            lg[:],
                            semb[:, dc, jc * 128:(jc + 1) * 128],
                            qemb[:, dc, isl],
                            start=(dc == 0), stop=(dc == DC - 1),
                        )
                    e = work.tile([128, 512], BF16, tag="e")
                    nc.scalar.activation(e[:], lg[:], FT.Exp)
                    nc.tensor.matmul(p_ps[:], oh_sb[:, jc, :], e[:],
                                     start=(jc == 0), stop=(jc == NJC - 1))
                pf = work.tile([C, 512], F32R, tag="pf")
                nc.scalar.copy(pf[:], p_ps[:])
                sum_ps = pssm.tile([1, 512], F32, tag="sum")
                nc.tensor.matmul(sum_ps[:], ones_col[:C],
                                 pf[:], start=True, stop=True)
                smr = work.tile([1, 512], F32R, tag="smr")
                nc.scalar.copy(smr[:], sum_ps[:])
                srep_ps = pssm.tile([C, 512], F32, tag="sumrep")
                nc.tensor.matmul(srep_ps[:], ones_row[:, :C],
                                 smr[:], start=True, stop=True)
                inv = work.tile([C, 512], F32, tag="inv")
                nc.vector.reciprocal(inv[:], srep_ps[:])
                o = work.tile([C, 512], F32, tag="o")
                nc.vector.tensor_mul(o[:], pf[:], inv[:])
                nc.sync.dma_start(out[:, isl], o[:])


_NC_CACHE = []


def _build():
    if _NC_CACHE:
        return _NC_CACHE[0]
    nc = bacc.Bacc("TRN2", target_bir_lowering=False, debug=False,
                   num_devices=N_CORES)
    s_t = nc.dram_tensor("s_t", [IND, NS], F32R, kind="ExternalInput").ap()
    q_t = nc.dram_tensor("q_t", [IND, NQC], F32R, kind="ExternalInput").ap()
    w = nc.dram_tensor("w", [IND, D], F32R, kind="ExternalInput").ap()
    b = nc.dram_tensor("b", [D], F32, kind="ExternalInput").ap()
    oh = nc.dram_tensor("oh", [NS, C], BF16, kind="ExternalInput").ap()
    out = nc.dram_tensor("out", [C, NQC], F32, kind="ExternalOutput").ap()
    with tile.TileContext(nc) as tc:
        _emit(nc, tc, s_t, q_t, w, b, oh, out)
    nc.compile()
    _NC_CACHE.append(nc)
    return nc


def _make_in_maps(support, query, W_enc, b_enc, support_labels):
    s_t = np.ascontiguousarray(np.asarray(support, dtype=np.float32).T)
    w = np.ascontiguousarray(np.asarray(W_enc, dtype=np.float32))
    b = np.ascontiguousarray(np.asarray(b_enc, dtype=np.float32))
    labels = np.asarray(support_labels).astype(np.int64)
    oh = np.zeros((NS, C), dtype=ml_dtypes.bfloat16)
    oh[np.arange(NS), labels] = 1
    q = np.asarray(query, dtype=np.float32)
    in_maps = []
    for i in range(N_CORES):
        q_t = np.ascontiguousarray(q[i * NQC:(i + 1) * NQC].T)
        in_maps.append({"s_t": s_t, "q_t": q_t, "w": w, "b": b, "oh": oh})
    return in_maps


def _run(in_maps, **kw):
    nc = _build()
    return bass_utils.run_bass_kernel_spmd(nc, in_maps,
                                           core_ids=list(range(N_CORES)), **kw)


def kernel(support, query, W_enc, b_enc, support_labels):
    in_maps = _make_in_maps(support, query, W_enc, b_enc, support_labels)
    res = _run(in_maps)
    return np.concatenate([res.results[i]["out"].T for i in range(N_CORES)],
                          axis=0)


# revision 5
# speedup vs baseline: 1.1439x; 1.1439x over previous
"""MatchingNetwork forward on 8 TRN2 NeuronCores.

Computation (reference):
    s_emb = l2norm(support @ W + b); q_emb = l2norm(query @ W + b)
    out = softmax(q_emb @ s_emb.T, axis=1) @ one_hot(labels, 64)

Strategy: data-parallel over query rows (1024/core), support replicated.
Host passes pre-transposed S^T / Q^T so every matmul has its contraction
dim on partitions (no on-device transposes). Attention is fused: per
512-query block we accumulate P^T[c,i] = sum_j OH[j,c]*exp(logit[j,i])
over support chunks; the softmax denominator is the column sum of P^T
(each one-hot row sums to 1), so the full attention matrix never exists.
"""

import sys

if "/opt/trn_rl_repo" not in sys.path:
    sys.path.insert(0, "/opt/trn_rl_repo")

import ml_dtypes
import numpy as np

import concourse.mybir as mybir
import concourse.tile as tile
from concourse import bacc, bass_utils

N_CORES = 8
NS, NQ, IND, D, C = 4096, 8192, 1024, 512, 64
NQC = NQ // N_CORES  # queries per core
KC = IND // 128      # 8 contraction chunks
DC = D // 128        # 4 embedding-dim chunks
JBLK = 512           # support/query columns per encode block
NJB = NS // JBLK     # 8 support encode blocks
NJC = NS // 128      # 32 support chunks in attention
NIB = NQC // 512     # 2 query blocks per core

F32 = mybir.dt.float32
F32R = mybir.dt.float32r
BF16 = mybir.dt.bfloat16


def _emit(nc, tc, s_t, q_t, w, b, oh, out):
    FT = mybir.ActivationFunctionType
    import contextlib

    with contextlib.ExitStack() as ctx:
        const = ctx.enter_context(tc.tile_pool(name="const", bufs=1))

        # Constants that need no DMA: build first so warmup matmuls can run
        # while the input DMAs stream in.
        ones_f32 = const.tile([128, 128], F32)
        nc.vector.memset(ones_f32[:], 1.0)
        ones_col = const.tile([128, 1], F32R)
        nc.scalar.copy(ones_col[:], ones_f32[:, 0:1])
        ones_row = const.tile([1, 128], F32R)
        nc.scalar.copy(ones_row[:], ones_f32[0:1, :])

        w_sb = const.tile([128, KC, D], F32R)
        wr = w.rearrange("(kc p) d -> p kc d", p=128)
        for kc in range(KC):  # split across DMA queues
            nc.sync.dma_start(w_sb[:, kc], wr[:, kc])
        b_sb = const.tile([128, DC], F32)
        nc.sync.dma_start(b_sb[:], b.rearrange("(dc p) -> p dc", p=128))
        oh_sb = const.tile([128, NJC, C], BF16)
        nc.sync.dma_start(oh_sb[:], oh.rearrange("(jc p) c -> p jc c", p=128))

        semb = const.tile([128, DC, NS], F32R)   # normalized S_emb^T, resident
        qemb = const.tile([128, DC, NQC], F32R)  # normalized Q_emb^T, resident

        # ~4us of tiny matmuls: warms the PE HAM clock gate to 2.4 GHz and
        # covers the initial input-DMA latency with PE activity.
        with tc.tile_pool(name="warm", bufs=1, space="PSUM") as warmp:
            wps = warmp.tile([1, 128], F32)
            for _ in range(24):
                nc.tensor.matmul(wps[:], ones_f32[:, 0:1], ones_f32[:],
                                 start=True, stop=True)

        def encode(x_t, n_cols, emb):
            """emb[:, dc, :] = l2norm-columns of (W^T @ x + b).

            The norm reduction for block jb is finished one block late so the
            PE never waits on ACT-produced squares."""
            with tc.tile_pool(name="enc_load", bufs=16) as loadp, \
                 tc.tile_pool(name="enc_work", bufs=8) as work, \
                 tc.tile_pool(name="enc_nw", bufs=2) as nwork, \
                 tc.tile_pool(name="enc_ps", bufs=3, space="PSUM") as psum, \
                 tc.tile_pool(name="enc_acc", bufs=2, space="PSUM") as psacc, \
                 tc.tile_pool(name="enc_rep", bufs=2, space="PSUM") as psrep:
                xr = x_t.rearrange("(kc p) n -> p kc n", p=128)
                nblk = n_cols // JBLK
                state = {}  # per-jb deferred norm state

                def finish_tail(jb):
                    # last norm matmul + the whole inv-norm chain for jb
                    js = slice(jb * JBLK, (jb + 1) * JBLK)
                    st = state.pop(jb)
                    nc.tensor.matmul(st["nrm"][:], ones_col[:],
                                     st["sq3"][:], start=False, stop=True)
                    nr = nwork.tile([1, JBLK], F32R, tag="nr")
                    nc.vector.tensor_copy(nr[:], st["nrm"][:])
                    rep_ps = psrep.tile([128, JBLK], F32, tag="rep")
                    nc.tensor.matmul(rep_ps[:], ones_row[:], nr[:],
                                     start=True, stop=True)
                    irec = nwork.tile([128, JBLK], F32, tag="irec")
                    nc.vector.reciprocal_approx_fast(irec[:], rep_ps[:])
                    isq = nwork.tile([128, JBLK], F32, tag="isq")
                    nc.scalar.activation(isq[:], irec[:], FT.Sqrt)
                    for dc in range(DC):
                        sl = emb[:, dc, js]
                        nc.vector.tensor_mul(sl, sl, isq[:])

                for jb in range(nblk):
                    js = slice(jb * JBLK, (jb + 1) * JBLK)
                    xt = []
                    for kc in range(KC):
                        t = loadp.tile([128, JBLK], F32R, tag="xt")
                        nc.sync.dma_start(t[:], xr[:, kc, js])
                        xt.append(t)
                    nrm_ps = psacc.tile([1, JBLK], F32, tag="nrm")
                    sqs = []
                    for dc in range(DC):
                        ps = psum.tile([128, JBLK], F32, tag="enc")
                        for kc in range(KC):
                            nc.tensor.matmul(
                                ps[:],
                                w_sb[:, kc, dc * 128:(dc + 1) * 128],
                                xt[kc][:],
                                start=(kc == 0), stop=(kc == KC - 1),
                            )
                        if dc == 1 and jb > 0:
                            finish_tail(jb - 1)
                        bias = b_sb[:, dc:dc + 1]
                        nc.scalar.activation(emb[:, dc, js], ps[:], FT.Identity,
                                             bias=bias)
                        sq = work.tile([128, JBLK], F32R, tag="sq")
                        nc.scalar.activation(sq[:], ps[:], FT.Square, bias=bias)
                        sqs.append(sq)
                        if dc >= 1:  # sq[dc-1] is ready; accumulate its norm
                            nc.tensor.matmul(nrm_ps[:], ones_col[:],
                                             sqs[dc - 1][:],
                                             start=(dc == 1), stop=False)
                    state[jb] = {"nrm": nrm_ps, "sq3": sqs[3]}
                finish_tail(nblk - 1)

        encode(q_t, NQC, qemb)   # small one first: cheaper DMA to wait on
        encode(s_t, NS, semb)

        with tc.tile_pool(name="att_work", bufs=4) as work, \
             tc.tile_pool(name="att_lg", bufs=3, space="PSUM") as pslg, \
             tc.tile_pool(name="att_acc", bufs=2, space="PSUM") as psacc, \
             tc.tile_pool(name="att_sm", bufs=1, space="PSUM") as pssm:
            for ib in range(NIB):
                isl = slice(ib * 512, (ib + 1) * 512)
                p_ps = psacc.tile([C, 512], F32, tag="pacc")
                pend = None  # deferred P-matmul: (e_tile, jc)
                for jc in range(NJC):
                    lg = pslg.tile([128, 512], F32, tag="lg")
                    for dc in range(DC):
                        nc.tensor.matmul(
                            lg[:],
                            semb[:, dc, jc * 128:(jc + 1) * 128],
                            qemb[:, dc, isl],
                            start=(dc == 0), stop=(dc == DC - 1),
                        )
                    if pend is not None:
                        e_prev, jp = pend
                        nc.tensor.matmul(p_ps[:], oh_sb[:, jp, :], e_prev[:],
                                         start=(jp == 0), stop=False)
                    e = work.tile([128, 512], BF16, tag="e")
                    nc.scalar.activation(e[:], lg[:], FT.Exp)
                    pend = (e, jc)
                e_prev, jp = pend
                nc.tensor.matmul(p_ps[:], oh_sb[:, jp, :], e_prev[:],
                                 start=False, stop=True)
                pf = work.tile([C, 512], F32R, tag="pf")
                nc.vector.tensor_copy(pf[:], p_ps[:])
                sum_ps = pssm.tile([1, 512], F32, tag="sum")
                nc.tensor.matmul(sum_ps[:], ones_col[:C],
                                 pf[:], start=True, stop=True)
                smr = work.tile([1, 512], F32R, tag="smr")
                nc.vector.tensor_copy(smr[:], sum_ps[:])
                srep_ps = pssm.tile([C, 512], F32, tag="sumrep")
                nc.tensor.matmul(srep_ps[:], ones_row[:, :C],
                                 smr[:], start=True, stop=True)
                inv = work.tile([C, 512], F32, tag="inv")
                nc.vector.reciprocal_approx_fast(inv[:], srep_ps[:])
                o = work.tile([C, 512], F32, tag="o")
                nc.vector.tensor_mul(o[:], pf[:], inv[:])
                nc.sync.dma_start(out[:, isl], o[:])


_NC_CACHE = []


def _build():
    if _NC_CACHE:
        return _NC_CACHE[0]
    nc = bacc.Bacc("TRN2", target_bir_lowering=False, debug=False,
                   num_devices=N_CORES)
    s_t = nc.dram_tensor("s_t", [IND, NS], F32R, kind="ExternalInput").ap()
    q_t = nc.dram_tensor("q_t", [IND, NQC], F32R, kind="ExternalInput").ap()
    w = nc.dram_tensor("w", [IND, D], F32R, kind="ExternalInput").ap()
    b = nc.dram_tensor("b", [D], F32, kind="ExternalInput").ap()
    oh = nc.dram_tensor("oh", [NS, C], BF16, kind="ExternalInput").ap()
    out = nc.dram_tensor("out", [C, NQC], F32, kind="ExternalOutput").ap()
    with tile.TileContext(nc) as tc:
        _emit(nc, tc, s_t, q_t, w, b, oh, out)
    nc.compile()
    _NC_CACHE.append(nc)
    return nc


def _make_in_maps(support, query, W_enc, b_enc, support_labels):
    s_t = np.ascontiguousarray(np.asarray(support, dtype=np.float32).T)
    w = np.ascontiguousarray(np.asarray(W_enc, dtype=np.float32))
    b = np.ascontiguousarray(np.asarray(b_enc, dtype=np.float32))
    labels = np.asarray(support_labels).astype(np.int64)
    oh = np.zeros((NS, C), dtype=ml_dtypes.bfloat16)
    oh[np.arange(NS), labels] = 1
    q = np.asarray(query, dtype=np.float32)
    in_maps = []
    for i in range(N_CORES):
        q_t = np.ascontiguousarray(q[i * NQC:(i + 1) * NQC].T)
        in_maps.append({"s_t": s_t, "q_t": q_t, "w": w, "b": b, "oh": oh})
    return in_maps


def _run(in_maps, **kw):
    nc = _build()
    return bass_utils.run_bass_kernel_spmd(nc, in_maps,
                                           core_ids=list(range(N_CORES)), **kw)


def kernel(support, query, W_enc, b_enc, support_labels):
    in_maps = _make_in_maps(support, query, W_enc, b_enc, support_labels)
    res = _run(in_maps)
    return np.concatenate([res.results[i]["out"].T for i in range(N_CORES)],
                          axis=0)


# revision 7
# speedup vs baseline: 1.2777x; 1.1170x over previous
"""MatchingNetwork forward on 8 TRN2 NeuronCores.

Computation (reference):
    s_emb = l2norm(support @ W + b); q_emb = l2norm(query @ W + b)
    out = softmax(q_emb @ s_emb.T, axis=1) @ one_hot(labels, 64)

Strategy: data-parallel over query rows (1024/core), support replicated.
Host passes pre-transposed S^T / Q^T so every matmul has its contraction
dim on partitions (no on-device transposes). Attention is fused: per
512-query block we accumulate P^T[c,i] = sum_j OH[j,c]*exp(logit[j,i])
over support chunks; the softmax denominator is the column sum of P^T
(each one-hot row sums to 1), so the full attention matrix never exists.
"""

import sys

if "/opt/trn_rl_repo" not in sys.path:
    sys.path.insert(0, "/opt/trn_rl_repo")

import ml_dtypes
import numpy as np

import concourse.mybir as mybir
import concourse.tile as tile
from concourse import bacc, bass_utils

N_CORES = 8
NS, NQ, IND, D, C = 4096, 8192, 1024, 512, 64
NQC = NQ // N_CORES  # queries per core
KC = IND // 128      # 8 contraction chunks
DC = D // 128        # 4 embedding-dim chunks
JBLK = 512           # support/query columns per encode block
NJB = NS // JBLK     # 8 support encode blocks
NJC = NS // 128      # 32 support chunks in attention
NIB = NQC // 512     # 2 query blocks per core
C2 = C + 1           # one-hot plus an all-ones denominator column

F32 = mybir.dt.float32
F32R = mybir.dt.float32r
BF16 = mybir.dt.bfloat16


def _emit(nc, tc, s_t, q_t, w, b, oh, out):
    FT = mybir.ActivationFunctionType
    import contextlib

    with contextlib.ExitStack() as ctx:
        const = ctx.enter_context(tc.tile_pool(name="const", bufs=1))

        # Constants that need no DMA: build first so warmup matmuls can run
        # while the input DMAs stream in.
        ones_f32 = const.tile([128, 128], F32)
        nc.vector.memset(ones_f32[:], 1.0)
        ones_col = const.tile([128, 1], F32R)
        nc.scalar.copy(ones_col[:], ones_f32[:, 0:1])
        ones_row = const.tile([1, 128], F32R)
        nc.scalar.copy(ones_row[:], ones_f32[0:1, :])

        wr = w.rearrange("(kc p) d -> p kc d", p=128)
        w_sb = []
        for kc in range(KC):  # separate tiles: fine-grained DMA deps
            t = const.tile([128, D], F32R, tag=f"w{kc}")
            (nc.sync if kc % 2 else nc.gpsimd).dma_start(t[:], wr[:, kc])
            w_sb.append(t)
        b_sb = const.tile([128, DC], F32)
        nc.gpsimd.dma_start(b_sb[:], b.rearrange("(dc p) -> p dc", p=128))
        oh_sb = const.tile([128, NJC, C2], BF16)
        nc.gpsimd.dma_start(oh_sb[:], oh.rearrange("(jc p) c -> p jc c", p=128))

        semb = const.tile([128, DC, NS], F32R)   # normalized S_emb^T, resident
        qemb = const.tile([128, DC, NQC], F32R)  # normalized Q_emb^T, resident

        # ~4us of tiny matmuls: warms the PE HAM clock gate to 2.4 GHz and
        # covers the initial input-DMA latency with PE activity.
        with tc.tile_pool(name="warm", bufs=1, space="PSUM") as warmp:
            wps = warmp.tile([1, 128], F32)
            for _ in range(32):
                nc.tensor.matmul(wps[:], ones_f32[:, 0:1], ones_f32[:],
                                 start=True, stop=True)

        def encode(x_t, n_cols, emb):
            """emb[:, dc, :] = l2norm-columns of (W^T @ x + b).

            The norm reduction for block jb is finished one block late so the
            PE never waits on ACT-produced squares."""
            with tc.tile_pool(name="enc_load", bufs=20) as loadp, \
                 tc.tile_pool(name="enc_work", bufs=8) as work, \
                 tc.tile_pool(name="enc_nw", bufs=2) as nwork, \
                 tc.tile_pool(name="enc_ps", bufs=3, space="PSUM") as psum, \
                 tc.tile_pool(name="enc_acc", bufs=2, space="PSUM") as psacc, \
                 tc.tile_pool(name="enc_rep", bufs=2, space="PSUM") as psrep:
                xr = x_t.rearrange("(kc p) n -> p kc n", p=128)
                nblk = n_cols // JBLK
                state = {}  # per-jb deferred norm state

                def finish_tail(jb):
                    # last norm matmul + the whole inv-norm chain for jb
                    js = slice(jb * JBLK, (jb + 1) * JBLK)
                    st = state.pop(jb)
                    nc.tensor.matmul(st["nrm"][:], ones_col[:],
                                     st["sq3"][:], start=False, stop=True)
                    nr = nwork.tile([1, JBLK], F32R, tag="nr")
                    nc.vector.tensor_copy(nr[:], st["nrm"][:])
                    rep_ps = psrep.tile([128, JBLK], F32, tag="rep")
                    nc.tensor.matmul(rep_ps[:], ones_row[:], nr[:],
                                     start=True, stop=True)
                    irec = nwork.tile([128, JBLK], F32, tag="irec")
                    nc.vector.reciprocal_approx_fast(irec[:], rep_ps[:])
                    isq = nwork.tile([128, JBLK], F32, tag="isq")
                    nc.scalar.activation(isq[:], irec[:], FT.Sqrt)
                    for dc in range(DC):
                        sl = emb[:, dc, js]
                        nc.vector.tensor_mul(sl, sl, isq[:])

                for jb in range(nblk):
                    js = slice(jb * JBLK, (jb + 1) * JBLK)
                    xt = []
                    for kc in range(KC):
                        t = loadp.tile([128, JBLK], F32R, tag="xt")
                        (nc.sync if kc % 2 else nc.gpsimd).dma_start(t[:], xr[:, kc, js])
                        xt.append(t)
                    nrm_ps = psacc.tile([1, JBLK], F32, tag="nrm")
                    sqs = []
                    for dc in range(DC):
                        ps = psum.tile([128, JBLK], F32, tag="enc")
                        for kc in range(KC):
                            nc.tensor.matmul(
                                ps[:],
                                w_sb[kc][:, dc * 128:(dc + 1) * 128],
                                xt[kc][:],
                                start=(kc == 0), stop=(kc == KC - 1),
                            )
                        if dc == 1 and jb > 0:
                            finish_tail(jb - 1)
                        bias = b_sb[:, dc:dc + 1]
                        nc.scalar.activation(emb[:, dc, js], ps[:], FT.Identity,
                                             bias=bias)
                        sq = work.tile([128, JBLK], F32R, tag="sq")
                        nc.scalar.activation(sq[:], ps[:], FT.Square, bias=bias)
                        sqs.append(sq)
                        if dc >= 1:  # sq[dc-1] is ready; accumulate its norm
                            nc.tensor.matmul(nrm_ps[:], ones_col[:],
                                             sqs[dc - 1][:],
                                             start=(dc == 1), stop=False)
                    state[jb] = {"nrm": nrm_ps, "sq3": sqs[3]}
                finish_tail(nblk - 1)

        encode(q_t, NQC, qemb)   # small one first: cheaper DMA to wait on
        encode(s_t, NS, semb)

        with tc.tile_pool(name="att_work", bufs=4) as work, \
             tc.tile_pool(name="att_lg", bufs=3, space="PSUM") as pslg, \
             tc.tile_pool(name="att_acc", bufs=2, space="PSUM") as psacc, \
             tc.tile_pool(name="att_sm", bufs=1, space="PSUM") as pssm:
            for ib in range(NIB):
                isl = slice(ib * 512, (ib + 1) * 512)
                p_ps = psacc.tile([C2, 512], F32, tag="pacc")
                pend = None  # deferred P-matmul: (e_tile, jc)
                for jc in range(NJC):
                    lg = pslg.tile([128, 512], F32, tag="lg")
                    for dc in range(DC):
                        nc.tensor.matmul(
                            lg[:],
                            semb[:, dc, jc * 128:(jc + 1) * 128],
                            qemb[:, dc, isl],
                            start=(dc == 0), stop=(dc == DC - 1),
                        )
                    if pend is not None:
                        e_prev, jp = pend
                        nc.tensor.matmul(p_ps[:], oh_sb[:, jp, :], e_prev[:],
                                         start=(jp == 0), stop=False)
                    e = work.tile([128, 512], BF16, tag="e")
                    nc.scalar.activation(e[:], lg[:], FT.Exp)
                    pend = (e, jc)
                e_prev, jp = pend
                nc.tensor.matmul(p_ps[:], oh_sb[:, jp, :], e_prev[:],
                                 start=False, stop=True)
                smr = work.tile([1, 512], F32R, tag="smr")
                nc.vector.tensor_copy(smr[:], p_ps[C:C + 1, :])
                srep_ps = pssm.tile([C, 512], F32, tag="sumrep")
                nc.tensor.matmul(srep_ps[:], ones_row[:, :C],
                                 smr[:], start=True, stop=True)
                inv = work.tile([C, 512], F32, tag="inv")
                nc.vector.reciprocal_approx_fast(inv[:], srep_ps[:])
                o = work.tile([C, 512], F32, tag="o")
                nc.vector.tensor_mul(o[:], p_ps[:C, :], inv[:])
                nc.sync.dma_start(out[:, isl], o[:])


_NC_CACHE = []


def _build():
    if _NC_CACHE:
        return _NC_CACHE[0]
    nc = bacc.Bacc("TRN2", target_bir_lowering=False, debug=False,
                   num_devices=N_CORES)
    s_t = nc.dram_tensor("s_t", [IND, NS], F32R, kind="ExternalInput").ap()
    q_t = nc.dram_tensor("q_t", [IND, NQC], F32R, kind="ExternalInput").ap()
    w = nc.dram_tensor("w", [IND, D], F32R, kind="ExternalInput").ap()
    b = nc.dram_tensor("b", [D], F32, kind="ExternalInput").ap()
    oh = nc.dram_tensor("oh", [NS, C2], BF16, kind="ExternalInput").ap()
    out = nc.dram_tensor("out", [C, NQC], F32, kind="ExternalOutput").ap()
    with tile.TileContext(nc) as tc:
        _emit(nc, tc, s_t, q_t, w, b, oh, out)
    nc.compile()
    _NC_CACHE.append(nc)
    return nc


def _make_in_maps(support, query, W_enc, b_enc, support_labels):
    s_t = np.ascontiguousarray(np.asarray(support, dtype=np.float32).T)
    w = np.ascontiguousarray(np.asarray(W_enc, dtype=np.float32))
    b = np.ascontiguousarray(np.asarray(b_enc, dtype=np.float32))
    labels = np.asarray(support_labels).astype(np.int64)
    oh = np.zeros((NS, C2), dtype=ml_dtypes.bfloat16)
    oh[np.arange(NS), labels] = 1
    oh[:, C] = 1
    q = np.asarray(query, dtype=np.float32)
    in_maps = []
    for i in range(N_CORES):
        q_t = np.ascontiguousarray(q[i * NQC:(i + 1) * NQC].T)
        in_maps.append({"s_t": s_t, "q_t": q_t, "w": w, "b": b, "oh": oh})
    return in_maps


def _run(in_maps, **kw):
    nc = _build()
    return bass_utils.run_bass_kernel_spmd(nc, in_maps,
                                           core_ids=list(range(N_CORES)), **kw)


def kernel(support, query, W_enc, b_enc, support_labels):
    in_maps = _make_in_maps(support, query, W_enc, b_enc, support_labels)
    res = _run(in_maps)
    return np.concatenate([res.results[i]["out"].T for i in range(N_CORES)],
                          axis=0)


# revision 9
# speedup vs baseline: 1.3536x; 1.0594x over previous
"""MatchingNetwork forward on 8 TRN2 NeuronCores.

Computation (reference):
    s_emb = l2norm(support @ W + b); q_emb = l2norm(query @ W + b)
    out = softmax(q_emb @ s_emb.T, axis=1) @ one_hot(labels, 64)

Strategy: data-parallel over query rows (1024/core), support replicated.
Host passes pre-transposed S^T / Q^T so every matmul has its contraction
dim on partitions (no on-device transposes). Attention is fused: per
512-query block we accumulate P^T[c,i] = sum_j OH[j,c]*exp(logit[j,i])
over support chunks; the softmax denominator is the column sum of P^T
(each one-hot row sums to 1), so the full attention matrix never exists.
"""

import sys

if "/opt/trn_rl_repo" not in sys.path:
    sys.path.insert(0, "/opt/trn_rl_repo")

import ml_dtypes
import numpy as np

import concourse.mybir as mybir
import concourse.tile as tile
from concourse import bacc, bass_utils

N_CORES = 8
NS, NQ, IND, D, C = 4096, 8192, 1024, 512, 64
NQC = NQ // N_CORES  # queries per core
KC = IND // 128      # 8 contraction chunks
DC = D // 128        # 4 embedding-dim chunks
JBLK = 512           # support/query columns per encode block
NJB = NS // JBLK     # 8 support encode blocks
NJC = NS // 128      # 32 support chunks in attention
NIB = NQC // 512     # 2 query blocks per core
C2 = C + 1           # one-hot plus an all-ones denominator column

F32 = mybir.dt.float32
F32R = mybir.dt.float32r
BF16 = mybir.dt.bfloat16


def _emit(nc, tc, s_t, q_t, w, b, oh, out):
    FT = mybir.ActivationFunctionType
    import contextlib

    with contextlib.ExitStack() as ctx:
        const = ctx.enter_context(tc.tile_pool(name="const", bufs=1))

        # Constants that need no DMA: build first so warmup matmuls can run
        # while the input DMAs stream in.
        ones_f32 = const.tile([128, 128], F32)
        nc.vector.memset(ones_f32[:], 1.0)
        ones_col = const.tile([128, 1], F32R)
        nc.scalar.copy(ones_col[:], ones_f32[:, 0:1])
        ones_row = const.tile([1, 128], F32R)
        nc.scalar.copy(ones_row[:], ones_f32[0:1, :])

        wr = w.rearrange("(kc p) d -> p kc d", p=128)
        w_sb = []
        for kc in range(KC):  # separate tiles: fine-grained DMA deps
            t = const.tile([128, D], BF16, tag=f"w{kc}")
            (nc.sync if kc % 2 else nc.gpsimd).dma_start(t[:], wr[:, kc])
            w_sb.append(t)
        b_sb = const.tile([128, DC], F32)
        nc.gpsimd.dma_start(b_sb[:], b.rearrange("(dc p) -> p dc", p=128))
        oh_sb = const.tile([128, NJC, C2], BF16)
        nc.gpsimd.dma_start(oh_sb[:], oh.rearrange("(jc p) c -> p jc c", p=128))

        # normalized embeddings, one resident tile per 512-column block so
        # attention's dependency tracking is per-block, not whole-tensor
        semb = [const.tile([128, DC, JBLK], F32R, tag=f"semb{i}", name=f"semb{i}")
                for i in range(NJB)]
        qemb = [const.tile([128, DC, JBLK], F32R, tag=f"qemb{i}", name=f"qemb{i}")
                for i in range(NIB)]

        # ~4us of tiny matmuls: warms the PE HAM clock gate to 2.4 GHz and
        # covers the initial input-DMA latency with PE activity.
        with tc.tile_pool(name="warm", bufs=1, space="PSUM") as warmp:
            wps = warmp.tile([1, 128], F32)
            for _ in range(32):
                nc.tensor.matmul(wps[:], ones_f32[:, 0:1], ones_f32[:],
                                 start=True, stop=True)

        def encode(x_t, n_cols, emb):
            """emb[:, dc, :] = l2norm-columns of (W^T @ x + b).

            The norm reduction for block jb is finished one block late so the
            PE never waits on ACT-produced squares."""
            with tc.tile_pool(name="enc_load", bufs=20) as loadp, \
                 tc.tile_pool(name="enc_work", bufs=8) as work, \
                 tc.tile_pool(name="enc_nw", bufs=2) as nwork, \
                 tc.tile_pool(name="enc_ps", bufs=3, space="PSUM") as psum, \
                 tc.tile_pool(name="enc_acc", bufs=2, space="PSUM") as psacc, \
                 tc.tile_pool(name="enc_rep", bufs=2, space="PSUM") as psrep:
                xr = x_t.rearrange("(kc p) n -> p kc n", p=128)
                nblk = n_cols // JBLK
                state = {}  # per-jb deferred norm state

                def finish_tail(jb):
                    # last norm matmul + the whole inv-norm chain for jb
                    st = state.pop(jb)
                    nc.tensor.matmul(st["nrm"][:], ones_col[:],
                                     st["sq3"][:], start=False, stop=True)
                    nr = nwork.tile([1, JBLK], F32R, tag="nr")
                    nc.vector.tensor_copy(nr[:], st["nrm"][:])
                    rep_ps = psrep.tile([128, JBLK], F32, tag="rep")
                    nc.tensor.matmul(rep_ps[:], ones_row[:], nr[:],
                                     start=True, stop=True)
                    irec = nwork.tile([128, JBLK], F32, tag="irec")
                    nc.vector.reciprocal_approx_fast(irec[:], rep_ps[:])
                    isq = nwork.tile([128, JBLK], F32, tag="isq")
                    nc.scalar.activation(isq[:], irec[:], FT.Sqrt)
                    for dc in range(DC):
                        sl = emb[jb][:, dc, :]
                        nc.vector.tensor_mul(sl, sl, isq[:])

                for jb in range(nblk):
                    js = slice(jb * JBLK, (jb + 1) * JBLK)
                    xt = []
                    for kc in range(KC):
                        t = loadp.tile([128, JBLK], BF16, tag="xt")
                        (nc.sync if kc % 2 else nc.gpsimd).dma_start(t[:], xr[:, kc, js])
                        xt.append(t)
                    nrm_ps = psacc.tile([1, JBLK], F32, tag="nrm")
                    sqs = []
                    for dc in range(DC):
                        ps = psum.tile([128, JBLK], F32, tag="enc")
                        for kc in range(KC):
                            nc.tensor.matmul(
                                ps[:],
                                w_sb[kc][:, dc * 128:(dc + 1) * 128],
                                xt[kc][:],
                                start=(kc == 0), stop=(kc == KC - 1),
                            )
                        if dc == 1 and jb > 0:
                            finish_tail(jb - 1)
                        bias = b_sb[:, dc:dc + 1]
                        nc.scalar.activation(emb[jb][:, dc, :], ps[:], FT.Identity,
                                             bias=bias)
                        sq = work.tile([128, JBLK], F32R, tag="sq")
                        nc.scalar.activation(sq[:], ps[:], FT.Square, bias=bias)
                        sqs.append(sq)
                        if dc >= 1:  # sq[dc-1] is ready; accumulate its norm
                            nc.tensor.matmul(nrm_ps[:], ones_col[:],
                                             sqs[dc - 1][:],
                                             start=(dc == 1), stop=False)
                    state[jb] = {"nrm": nrm_ps, "sq3": sqs[3]}
                finish_tail(nblk - 1)

        encode(q_t, NQC, qemb)   # small one first: cheaper DMA to wait on
        encode(s_t, NS, semb)

        with tc.tile_pool(name="att_work", bufs=4) as work, \
             tc.tile_pool(name="att_lg", bufs=3, space="PSUM") as pslg, \
             tc.tile_pool(name="att_acc", bufs=2, space="PSUM") as psacc, \
             tc.tile_pool(name="att_sm", bufs=1, space="PSUM") as pssm:
            for ib in range(NIB):
                isl = slice(ib * 512, (ib + 1) * 512)
                p_ps = psacc.tile([C2, 512], F32, tag="pacc")
                pend = None  # deferred P-matmul: (e_tile, jc)
                for jc in range(NJC):
                    lg = pslg.tile([128, 512], F32, tag="lg")
                    for dc in range(DC):
                        nc.tensor.matmul(
                            lg[:],
                            semb[jc // 4][:, dc, (jc % 4) * 128:(jc % 4 + 1) * 128],
                            qemb[ib][:, dc, :],
                            start=(dc == 0), stop=(dc == DC - 1),
                        )
                    if pend is not None:
                        e_prev, jp = pend
                        nc.tensor.matmul(p_ps[:], oh_sb[:, jp, :], e_prev[:],
                                         start=(jp == 0), stop=False)
                    e = work.tile([128, 512], BF16, tag="e")
                    nc.scalar.activation(e[:], lg[:], FT.Exp)
                    pend = (e, jc)
                e_prev, jp = pend
                nc.tensor.matmul(p_ps[:], oh_sb[:, jp, :], e_prev[:],
                                 start=False, stop=True)
                smr = work.tile([1, 512], F32R, tag="smr")
                nc.vector.tensor_copy(smr[:], p_ps[C:C + 1, :])
                srep_ps = pssm.tile([C, 512], F32, tag="sumrep")
                nc.tensor.matmul(srep_ps[:], ones_row[:, :C],
                                 smr[:], start=True, stop=True)
                inv = work.tile([C, 512], F32, tag="inv")
                nc.vector.reciprocal_approx_fast(inv[:], srep_ps[:])
                o = work.tile([C, 512], F32, tag="o")
                nc.vector.tensor_mul(o[:], p_ps[:C, :], inv[:])
                nc.sync.dma_start(out[:, isl], o[:])


_NC_CACHE = []


def _build():
    if _NC_CACHE:
        return _NC_CACHE[0]
    nc = bacc.Bacc("TRN2", target_bir_lowering=False, debug=False,
                   num_devices=N_CORES)
    s_t = nc.dram_tensor("s_t", [IND, NS], BF16, kind="ExternalInput").ap()
    q_t = nc.dram_tensor("q_t", [IND, NQC], BF16, kind="ExternalInput").ap()
    w = nc.dram_tensor("w", [IND, D], BF16, kind="ExternalInput").ap()
    b = nc.dram_tensor("b", [D], F32, kind="ExternalInput").ap()
    oh = nc.dram_tensor("oh", [NS, C2], BF16, kind="ExternalInput").ap()
    out = nc.dram_tensor("out", [C, NQC], F32, kind="ExternalOutput").ap()
    with tile.TileContext(nc) as tc:
        _emit(nc, tc, s_t, q_t, w, b, oh, out)
    nc.compile()
    _NC_CACHE.append(nc)
    return nc


def _make_in_maps(support, query, W_enc, b_enc, support_labels):
    s_t = np.ascontiguousarray(np.asarray(support, dtype=np.float32).T).astype(ml_dtypes.bfloat16)
    w = np.ascontiguousarray(np.asarray(W_enc, dtype=np.float32)).astype(ml_dtypes.bfloat16)
    b = np.ascontiguousarray(np.asarray(b_enc, dtype=np.float32))
    labels = np.asarray(support_labels).astype(np.int64)
    oh = np.zeros((NS, C2), dtype=ml_dtypes.bfloat16)
    oh[np.arange(NS), labels] = 1
    oh[:, C] = 1
    q = np.asarray(query, dtype=np.float32)
    in_maps = []
    for i in range(N_CORES):
        q_t = np.ascontiguousarray(q[i * NQC:(i + 1) * NQC].T).astype(ml_dtypes.bfloat16)
        in_maps.append({"s_t": s_t, "q_t": q_t, "w": w, "b": b, "oh": oh})
    return in_maps


def _run(in_maps, **kw):
    nc = _build()
    return bass_utils.run_bass_kernel_spmd(nc, in_maps,
                                           core_ids=list(range(N_CORES)), **kw)


def kernel(support, query, W_enc, b_enc, support_labels):
    in_maps = _make_in_maps(support, query, W_enc, b_enc, support_labels)
    res = _run(in_maps)
    return np.concatenate([res.results[i]["out"].T for i in range(N_CORES)],
                          axis=0)
